# revision 17
# baseline (speedup 1.0000x reference)
"""Multi-head self-attention with RoPE on 8 TRN2 NeuronCores.

Sharding: core c = (b, hg): b = c // 4 (data parallel over batch),
hg = c % 4 (tensor parallel over head groups of 4 heads = 512 features).
Each core computes q/k/v projections for its 4 heads, RoPE, causal
attention, and a partial out-projection [S, E] in bf16; the host sums
the 4 partials per batch and adds bo.

Performance scheme (single pass over all 4 heads):
- q/k/v and out projections run as fp8e4m3 DoubleRow matmuls (0.5
  cycles/row, 2x128 contraction per instruction) with a 3-product
  hi/lo residual split (x_hi@W_hi + x_lo@W_hi + x_hi@W_lo) that keeps
  quantization error at the ~1e-3 level: 0.75 cycles per fp32r-row
  equivalent. x and W splits are precomputed on the host; the
  attention-output split is computed on device (ACT cast + DVE sub).
- Attention (scores, attn@V) runs in bf16 (1 cycle/row, exact f32
  accumulation in PSUM). RoPE runs as 3 DVE scalar_tensor_tensor ops
  per tensor (full-width cos term + two half-partition swapped sin
  terms) with the 1/1024 projection descale folded into the bf16
  cos/sin tables, combining on the Pool engine.
- The softmax denominator accumulates in f32 from bf16 chunk-pair sums
  (DVE), is partition-reduced on Pool, and reciprocal'd on DVE. Causal
  masks multiply on Pool. exp runs on ACT writing bf16.
- Diagonal k-chunks compute only from the covering pair start so
  chunk-pair ops (dacc) see fully-masked zeros in the extension.
"""

import sys

if "/opt/trn_rl_repo" not in sys.path:
    sys.path.insert(0, "/opt/trn_rl_repo")

import numpy as np
import ml_dtypes

import concourse.bass as bass  # noqa: F401
import concourse.mybir as mybir
from concourse import bacc
from concourse.tile import TileContext
from concourse.bass_utils import run_bass_kernel_spmd

B, S, E, H, D = 2, 2048, 2048, 16, 128
NCORES = 8
GROUPS = 4          # head groups (tensor parallel)
HPC = H // GROUPS   # heads per core (4)
FH = HPC * D        # features per core (512)
ECH = E // 128      # contraction chunks (16)
NPAIR = ECH // 2    # DoubleRow chunk pairs (8)
SB = 512            # s-block width
NSB = S // SB       # 4 s-blocks
NST = S // 128      # 16 s chunks

SX = 16.0           # fp8 scale on x
SW = 64.0           # fp8 scale on weights
SAO = 16.0          # fp8 scale on attention output
PRJ = SX * SW       # projection psum scale (1024)

dt = mybir.dt
F32 = dt.float32
BF16 = dt.bfloat16
F8 = dt.float8e4
AX = mybir.AluOpType
ACTF = mybir.ActivationFunctionType
DR = mybir.MatmulPerfMode.DoubleRow
F8NP = ml_dtypes.float8_e4m3
BFNP = ml_dtypes.bfloat16

_CACHE = {}


def _build_program():
    nc = bacc.Bacc("TRN2", target_bir_lowering=False, debug=False,
                   num_devices=NCORES)

    xhi_d = nc.dram_tensor("xhi", [128, ECH, S], F8, kind="ExternalInput")
    xlo_d = nc.dram_tensor("xlo", [128, ECH, S], F8, kind="ExternalInput")
    w_d = {}
    for nm in ("wq", "wk", "wv"):
        for hl in ("hi", "lo"):
            w_d[nm + hl] = nc.dram_tensor(nm + hl, [128, ECH, FH], F8,
                                          kind="ExternalInput")
    wohi_d = nc.dram_tensor("wohi", [128, HPC, E], F8, kind="ExternalInput")
    wolo_d = nc.dram_tensor("wolo", [128, HPC, E], F8, kind="ExternalInput")
    bqk_d = nc.dram_tensor("bqk", [128, 4 * HPC], F32, kind="ExternalInput")
    bv_d = nc.dram_tensor("bv_rep", [128, FH], BF16, kind="ExternalInput")
    cos_d = nc.dram_tensor("cos_t", [128, S], BF16, kind="ExternalInput")
    sin_d = nc.dram_tensor("sin_t", [128, S], BF16, kind="ExternalInput")
    cmask_d = nc.dram_tensor("cmask", [128, 4 * SB], BF16,
                             kind="ExternalInput")
    out_d = nc.dram_tensor("out", [S, E], BF16, kind="ExternalOutput")

    inv_sqrt_d = float(1.0 / np.sqrt(D))

    with TileContext(nc) as tc:
        with (
            tc.tile_pool(name="psum", bufs=2, space="PSUM") as psp,
            tc.tile_pool(name="cst", bufs=1) as cst,
            tc.tile_pool(name="wp", bufs=1) as wp,
            tc.tile_pool(name="kv", bufs=1) as kvp,
            tc.tile_pool(name="xp", bufs=2) as xp,
            tc.tile_pool(name="st", bufs=2) as st1,
            tc.tile_pool(name="aop", bufs=2) as aop,
            tc.tile_pool(name="osp", bufs=2) as osp,
        ):
            cm_t = cst.tile([128, 4 * SB], BF16, tag="cm")
            bqk_t = cst.tile([128, 4 * HPC], F32, tag="bqk")
            bv_t = cst.tile([128, FH], BF16, tag="bv")
            cos_t = cst.tile([128, S], BF16, tag="cos")
            sin_t = cst.tile([128, S], BF16, tag="sin")
            wu_t = cst.tile([128, SB], BF16, tag="wu")
            nc.any.memset(wu_t[:], 0.5)

            def load_x(sb):
                ssl = slice(sb * SB, (sb + 1) * SB)
                xh = xp.tile([128, ECH, SB], F8, tag="xh", name="xh")
                xl = xp.tile([128, ECH, SB], F8, tag="xl", name="xl")
                nc.sync.dma_start(out=xh[:], in_=xhi_d[:, :, ssl])
                nc.sync.dma_start(out=xl[:], in_=xlo_d[:, :, ssl])
                return xh, xl

            # startup order: the first q chain needs x_hi + wq_hi first,
            # then x_lo + wq_lo (products 2/3), then rope tables.
            ssl0 = slice(0, SB)
            xh0 = xp.tile([128, ECH, SB], F8, tag="xh", name="xh0")
            xl0 = xp.tile([128, ECH, SB], F8, tag="xl", name="xl0")
            nc.sync.dma_start(out=xh0[:], in_=xhi_d[:, :, ssl0])
            wt = {}
            for nm in ("wq", "wk", "wv"):
                for hl in ("hi", "lo"):
                    wt[nm + hl] = wp.tile([128, ECH, FH], F8, tag=nm + hl,
                                          name=nm + hl)
            nc.sync.dma_start(out=wt["wqhi"][:], in_=w_d["wqhi"][:])
            nc.sync.dma_start(out=xl0[:], in_=xlo_d[:, :, ssl0])
            nc.sync.dma_start(out=wt["wqlo"][:], in_=w_d["wqlo"][:])
            xs0 = (xh0, xl0)
            nc.sync.dma_start(out=bqk_t[:], in_=bqk_d[:])
            nc.sync.dma_start(out=cos_t[:], in_=cos_d[:])
            nc.sync.dma_start(out=sin_t[:], in_=sin_d[:])
            for nm in ("wk", "wv"):
                for hl in ("hi", "lo"):
                    nc.sync.dma_start(out=wt[nm + hl][:], in_=w_d[nm + hl][:])
            nc.sync.dma_start(out=bv_t[:], in_=bv_d[:])
            nc.sync.dma_start(out=cm_t[:], in_=cmask_d[:])
            wo_hi = wp.tile([128, HPC, E], F8, tag="wohi")
            wo_lo = wp.tile([128, HPC, E], F8, tag="wolo")
            nc.sync.dma_start(out=wo_hi[:], in_=wohi_d[:])
            nc.sync.dma_start(out=wo_lo[:], in_=wolo_d[:])

            # persistent k (per head, [d, S]) and v ([s128, (chunk, h, d)])
            kh = [kvp.tile([128, S], BF16, tag=f"kh{h}", name=f"kh{h}")
                  for h in range(HPC)]
            vh = kvp.tile([128, NST, FH], BF16, tag="vh")

            # PE warm-up: lifts the clock gate while initial DMAs fill
            # (memset input so no DMA dependency).
            pwarm = psp.tile([128, SB], F32, tag="po", bufs=2, name="pwarm")
            for i in range(24):
                nc.tensor.matmul(pwarm[:], wu_t[:, 0:128], wu_t[:, 0:SB],
                                 start=(i == 0), stop=(i == 23))

            def emit_qk(sb, h, xs):
                """Project+rope q and k for head h of s-block sb."""
                xh, xl = xs
                ssl = slice(sb * SB, (sb + 1) * SB)
                fsl = slice(h * 128, (h + 1) * 128)
                qtile = None
                for kind in ("q", "k"):
                    whi = wt[("wq" if kind == "q" else "wk") + "hi"]
                    wlo = wt[("wq" if kind == "q" else "wk") + "lo"]
                    ps = psp.tile([128, SB], F32, tag="ps1", bufs=3,
                                  name="psqk")
                    for j in range(NPAIR):
                        jp = slice(2 * j, 2 * j + 2)
                        nc.tensor.matmul(ps[:], whi[:, jp, fsl], xh[:, jp, :],
                                         start=(j == 0), stop=False,
                                         perf_mode=DR)
                    for j in range(NPAIR):
                        jp = slice(2 * j, 2 * j + 2)
                        nc.tensor.matmul(ps[:], whi[:, jp, fsl], xl[:, jp, :],
                                         start=False, stop=False,
                                         perf_mode=DR)
                    for j in range(NPAIR):
                        jp = slice(2 * j, 2 * j + 2)
                        nc.tensor.matmul(ps[:], wlo[:, jp, fsl], xh[:, jp, :],
                                         start=False, stop=(j == NPAIR - 1),
                                         perf_mode=DR)
                    bofs = 0 if kind == "q" else 2 * HPC
                    bias = bqk_t[:, bofs + h:bofs + h + 1]
                    bias_sw = bqk_t[:, bofs + HPC + h:bofs + HPC + h + 1]
                    t1 = st1.tile([128, SB], BF16, tag="t1", bufs=2)
                    nc.vector.scalar_tensor_tensor(
                        out=t1[:], in0=ps[:], scalar=bias,
                        in1=cos_t[:, ssl], op0=AX.add, op1=AX.mult)
                    t2 = st1.tile([128, SB], BF16, tag="t2", bufs=2)
                    nc.vector.scalar_tensor_tensor(
                        out=t2[0:64, :], in0=ps[64:128, :],
                        scalar=bias_sw[0:64], in1=sin_t[0:64, ssl],
                        op0=AX.add, op1=AX.mult)
                    nc.vector.scalar_tensor_tensor(
                        out=t2[64:128, :], in0=ps[0:64, :],
                        scalar=bias_sw[64:128], in1=sin_t[64:128, ssl],
                        op0=AX.add, op1=AX.mult)
                    if kind == "q":
                        dst = st1.tile([128, SB], BF16, tag="qh", bufs=8,
                                       name="qh")
                        qtile = dst
                        dview = dst[:]
                    else:
                        dview = kh[h][:, ssl]
                    nc.gpsimd.tensor_add(dview, t1[:], t2[:])
                return qtile

            def emit_v(sb, xs):
                xh, xl = xs
                for ssub in range(SB // 128):
                    scol = slice(ssub * 128, (ssub + 1) * 128)
                    ps = psp.tile([128, FH], F32, tag="ps1", bufs=3,
                                  name="psv")
                    for j in range(NPAIR):
                        jp = slice(2 * j, 2 * j + 2)
                        nc.tensor.matmul(ps[:], xh[:, jp, scol],
                                         wt["wvhi"][:, jp, :],
                                         start=(j == 0), stop=False,
                                         perf_mode=DR)
                    for j in range(NPAIR):
                        jp = slice(2 * j, 2 * j + 2)
                        nc.tensor.matmul(ps[:], xl[:, jp, scol],
                                         wt["wvhi"][:, jp, :],
                                         start=False, stop=False,
                                         perf_mode=DR)
                    for j in range(NPAIR):
                        jp = slice(2 * j, 2 * j + 2)
                        nc.tensor.matmul(ps[:], xh[:, jp, scol],
                                         wt["wvlo"][:, jp, :],
                                         start=False, stop=(j == NPAIR - 1),
                                         perf_mode=DR)
                    # vh = ps/PRJ + bv  (bf16)
                    nc.vector.scalar_tensor_tensor(
                        out=vh[:, sb * 4 + ssub, :], in0=ps[:],
                        scalar=float(1.0 / PRJ), in1=bv_t[:],
                        op0=AX.mult, op1=AX.add)

            def attn_gen(sb, h, qtile, aohi, aolo):
                """Causal attention q-tile sb for head h (bf16); generator
                yielding once per k-chunk so two heads can interleave."""
                nk = (sb + 1) * 4
                po = psp.tile([128, SB], F32, tag="po", bufs=2, name="po")
                dacc = st1.tile([128, SB], F32, tag="dacc", bufs=3)
                pexps = []      # (ki, pexp, q0)
                pending = []
                for ki in range(nk):
                    j = ki - sb * 4
                    # diagonal chunks start at their pair's q0 so pair ops
                    # see fully-masked zeros in the extension
                    q0 = 128 * (j - (j % 2)) if j > 0 else 0
                    ksl = slice(ki * 128, (ki + 1) * 128)
                    pscore = psp.tile([128, SB], F32, tag="pscore",
                                      bufs=3, name="pscore")
                    nc.tensor.matmul(pscore[:, q0:SB], kh[h][:, ksl],
                                     qtile[:, q0:SB], start=True, stop=True)
                    pexp = st1.tile([128, SB], BF16, tag="pexp", bufs=10)
                    nc.scalar.activation(pexp[:, q0:SB], pscore[:, q0:SB],
                                         ACTF.Exp, scale=inv_sqrt_d)
                    if j >= 0:
                        nc.vector.tensor_mul(
                            pexp[:, q0:SB], pexp[:, q0:SB],
                            cm_t[:, j * SB + q0:(j + 1) * SB])
                    pexps.append((ki, pexp, q0))
                    if ki % 2 == 1:
                        _, pa, pq0 = pexps[ki - 1]
                        if ki == 1:
                            nc.vector.tensor_add(dacc[:, pq0:SB],
                                                 pa[:, pq0:SB],
                                                 pexp[:, pq0:SB])
                        else:
                            sp = st1.tile([128, SB], BF16, tag="spair",
                                          bufs=3)
                            nc.vector.tensor_add(sp[:, pq0:SB],
                                                 pa[:, pq0:SB],
                                                 pexp[:, pq0:SB])
                            nc.vector.tensor_add(dacc[:, pq0:SB],
                                                 dacc[:, pq0:SB],
                                                 sp[:, pq0:SB])
                    pending.append((ki, pexp, q0))
                    if len(pending) > 2:
                        k0, px, pq = pending.pop(0)
                        nc.tensor.matmul(
                            po[:, pq:SB],
                            vh[:, k0, h * 128:(h + 1) * 128],
                            px[:, pq:SB], start=(k0 == 0), stop=False)
                    yield
                while pending:
                    k0, px, pq = pending.pop(0)
                    last = not pending
                    nc.tensor.matmul(po[:, pq:SB],
                                     vh[:, k0, h * 128:(h + 1) * 128],
                                     px[:, pq:SB], start=(k0 == 0), stop=last)
                dred = st1.tile([128, SB], F32, tag="dred", bufs=3)
                nc.gpsimd.partition_all_reduce(
                    out_ap=dred[:], in_ap=dacc[:], channels=128,
                    reduce_op=__import__("concourse.bass_isa",
                                         fromlist=["ReduceOp"]).ReduceOp.add)
                rec = st1.tile([128, SB], F32, tag="rec", bufs=3)
                nc.vector.reciprocal(rec[:], dred[:])
                t = st1.tile([128, SB], BF16, tag="taot", bufs=3)
                nc.vector.scalar_tensor_tensor(
                    out=t[:], in0=po[:], scalar=SAO, in1=rec[:],
                    op0=AX.mult, op1=AX.mult)
                nc.scalar.copy(aohi[:, h, :], t[:])
                nc.vector.tensor_sub(aolo[:, h, :], t[:], aohi[:, h, :])

            def emit_outproj_piece(sb, sti, aohi, aolo):
                """Out-projection for one 128-row s-chunk of s-block sb.
                Head-pair-major chain order so the first half only depends
                on heads 0/1."""
                stsl = slice(sti * 128, (sti + 1) * 128)
                osb = osp.tile([128, E], BF16, tag="osb", name="osb")
                for gt in range(E // 512):
                    gsl = slice(gt * 512, (gt + 1) * 512)
                    psO = psp.tile([128, 512], F32, tag="ps1", bufs=3,
                                   name="psO")
                    n = 0
                    for j in range(HPC // 2):
                        jp = slice(2 * j, 2 * j + 2)
                        for lhs, rhs in ((aohi, wo_hi), (aolo, wo_hi),
                                         (aohi, wo_lo)):
                            nc.tensor.matmul(psO[:], lhs[:, jp, stsl],
                                             rhs[:, jp, gsl],
                                             start=(n == 0), stop=(n == 5),
                                             perf_mode=DR)
                            n += 1
                    nc.scalar.activation(osb[:, gsl], psO[:], ACTF.Copy,
                                         scale=float(1.0 / (SAO * SW)))
                row0 = sb * SB + sti * 128
                nc.sync.dma_start(out=out_d[row0:row0 + 128, :],
                                  in_=osb[:])

            def drive_pair(ga, gb):
                """Round-robin two attention generators."""
                live = [ga, gb]
                while live:
                    for g in list(live):
                        try:
                            next(g)
                        except StopIteration:
                            live.remove(g)

            # ---- stage loop: attn(sb) head-pairs interleaved, plus
            # proj(sb+1) and out-projection pieces of sb-1 ----
            q_cur = [emit_qk(0, h, xs0) for h in range(HPC)]
            emit_v(0, xs0)
            prev = None     # (sb-1, aohi, aolo) with outproj still pending
            for sb in range(NSB):
                nxt = sb + 1
                if nxt < NSB:
                    xs_n = load_x(nxt)
                aohi = aop.tile([128, HPC, SB], F8, tag="aohi", name="aohi")
                aolo = aop.tile([128, HPC, SB], F8, tag="aolo", name="aolo")
                q_next = []
                for hp in (0, 2):
                    drive_pair(attn_gen(sb, hp, q_cur[hp], aohi, aolo),
                               attn_gen(sb, hp + 1, q_cur[hp + 1],
                                        aohi, aolo))
                    for h in (hp, hp + 1):
                        if nxt < NSB:
                            q_next.append(emit_qk(nxt, h, xs_n))
                        if prev is not None:
                            emit_outproj_piece(prev[0], h, prev[1], prev[2])
                if nxt < NSB:
                    emit_v(nxt, xs_n)
                prev = (sb, aohi, aolo)
                q_cur = q_next
            for sti in range(SB // 128):
                emit_outproj_piece(prev[0], sti, prev[1], prev[2])

    nc.compile()
    return nc


def _host_constants():
    """RoPE cos/sin tables (evens-first, pre-descaled) and causal masks."""
    i = np.arange(64, dtype=np.float64)
    freqs = np.power(10000.0, -2.0 * i / D)
    pos = np.arange(S, dtype=np.float64)
    ang = pos[None, :] * freqs[:, None]              # [64, S]
    cos = np.cos(ang)
    sin = np.sin(ang)
    cos_t = (np.concatenate([cos, cos], axis=0) / PRJ).astype(BFNP)
    sin_t = (np.concatenate([-sin, sin], axis=0) / PRJ).astype(BFNP)
    r = np.arange(128)[:, None]
    c = np.arange(SB)[None, :]
    masks = [(128 * j + r <= c).astype(np.float32) for j in range(4)]
    cmask = np.concatenate(masks, axis=1).astype(BFNP)
    return cos_t, sin_t, cmask


def _split8(t, s):
    hi = (s * t).astype(F8NP)
    lo = (s * t - hi.astype(np.float32)).astype(F8NP)
    return hi, lo


def _chunked(t, nch):
    """[nch*128, N] f8 -> [128, nch, N]"""
    n = t.shape[1]
    return np.ascontiguousarray(
        t.reshape(nch, 128, n).transpose(1, 0, 2))


def kernel(x, Wq, bq, Wk, bk, Wv, bv, Wo, bo):
    x = np.asarray(x, dtype=np.float32)
    Wq = np.asarray(Wq, dtype=np.float32)
    bq = np.asarray(bq, dtype=np.float32)
    Wk = np.asarray(Wk, dtype=np.float32)
    bk = np.asarray(bk, dtype=np.float32)
    Wv = np.asarray(Wv, dtype=np.float32)
    bv = np.asarray(bv, dtype=np.float32)
    Wo = np.asarray(Wo, dtype=np.float32)
    bo = np.asarray(bo, dtype=np.float32)

    if "nc" not in _CACHE:
        _CACHE["nc"] = _build_program()
        _CACHE["consts"] = _host_constants()
    nc = _CACHE["nc"]
    cos_t, sin_t, cmask = _CACHE["consts"]

    perm = np.concatenate([np.arange(0, D, 2), np.arange(1, D, 2)])
    sw64 = np.concatenate([np.arange(64, 128), np.arange(0, 64)])

    xsplit = []
    for b in range(B):
        xT = np.ascontiguousarray(x[b].T)
        xh, xl = _split8(xT, SX)
        xsplit.append((_chunked(xh, ECH), _chunked(xl, ECH)))

    in_maps = []
    for c in range(NCORES):
        b, hg = divmod(c, GROUPS)
        rows = slice(hg * FH, (hg + 1) * FH)
        Wq_s = Wq[rows].reshape(HPC, D, E)[:, perm, :].reshape(FH, E)
        Wk_s = Wk[rows].reshape(HPC, D, E)[:, perm, :].reshape(FH, E)
        bq_s = bq[rows].reshape(HPC, D)[:, perm]     # [HPC, 128]
        bk_s = bk[rows].reshape(HPC, D)[:, perm]
        bqk_t = PRJ * np.concatenate(
            [bq_s, bq_s[:, sw64], bk_s, bk_s[:, sw64]],
            axis=0).T.astype(np.float32)
        bqk_t = np.ascontiguousarray(bqk_t)          # [128, 4*HPC]

        wqh, wql = _split8(np.ascontiguousarray(Wq_s.T), SW)
        wkh, wkl = _split8(np.ascontiguousarray(Wk_s.T), SW)
        wvh, wvl = _split8(np.ascontiguousarray(Wv[rows].T), SW)
        woh, wol = _split8(np.ascontiguousarray(Wo[:, rows].T), SW)

        in_maps.append({
            "xhi": xsplit[b][0],
            "xlo": xsplit[b][1],
            "wqhi": _chunked(wqh, ECH), "wqlo": _chunked(wql, ECH),
            "wkhi": _chunked(wkh, ECH), "wklo": _chunked(wkl, ECH),
            "wvhi": _chunked(wvh, ECH), "wvlo": _chunked(wvl, ECH),
            "wohi": _chunked(woh, HPC), "wolo": _chunked(wol, HPC),
            "bqk": bqk_t,
            "bv_rep": np.ascontiguousarray(
                np.broadcast_to(bv[rows], (128, FH))).astype(BFNP),
            "cos_t": cos_t,
            "sin_t": sin_t,
            "cmask": cmask,
        })

    res = run_bass_kernel_spmd(nc, in_maps, list(range(NCORES)))
    outs = [res.results[c]["out"] for c in range(NCORES)]

    result = np.empty((B, S, E), dtype=np.float32)
    for b in range(B):
        acc = outs[GROUPS * b].astype(np.float32)
        for g in range(1, GROUPS):
            acc = acc + outs[GROUPS * b + g].astype(np.float32)
        result[b] = acc + bo[None, :]
    return result


# revision 37
# speedup vs baseline: 1.0707x; 1.0707x over previous
"""Multi-head self-attention with RoPE on 8 TRN2 NeuronCores.

Sharding: core c = (b, hg): b = c // 4 (data parallel over batch),
hg = c % 4 (tensor parallel over head groups of 4 heads = 512 features).
Each core computes q/k/v projections for its 4 heads, RoPE, causal
attention, and a partial out-projection [S, E] in bf16; the host sums
the 4 partials per batch and adds bo.

Performance scheme (single pass over all 4 heads):
- q/k/v and out projections run as fp8e4m3 DoubleRow matmuls (0.5
  cycles/row, 2x128 contraction per instruction) with a 3-product
  hi/lo residual split (x_hi@W_hi + x_lo@W_hi + x_hi@W_lo) that keeps
  quantization error at the ~1e-3 level: 0.75 cycles per fp32r-row
  equivalent. x and W splits are precomputed on the host; the
  attention-output split is computed on device (ACT cast + DVE sub).
- Attention (scores, attn@V) runs in bf16 (1 cycle/row, exact f32
  accumulation in PSUM). RoPE runs as 3 DVE scalar_tensor_tensor ops
  per tensor (full-width cos term + two half-partition swapped sin
  terms) with the 1/1024 projection descale folded into the bf16
  cos/sin tables, combining on the Pool engine.
- The softmax denominator accumulates in f32 from bf16 chunk-pair sums
  (DVE), is partition-reduced on Pool, and reciprocal'd on DVE. Causal
  masks multiply on Pool. exp runs on ACT writing bf16.
- Diagonal k-chunks compute only from the covering pair start so
  chunk-pair ops (dacc) see fully-masked zeros in the extension.
"""

import sys

if "/opt/trn_rl_repo" not in sys.path:
    sys.path.insert(0, "/opt/trn_rl_repo")

import numpy as np
import ml_dtypes

import concourse.bass as bass  # noqa: F401
import concourse.mybir as mybir
from concourse import bacc
from concourse.tile import TileContext
from concourse.bass_utils import run_bass_kernel_spmd

B, S, E, H, D = 2, 2048, 2048, 16, 128
NCORES = 8
GROUPS = 4          # head groups (tensor parallel)
HPC = H // GROUPS   # heads per core (4)
FH = HPC * D        # features per core (512)
ECH = E // 128      # contraction chunks (16)
NPAIR = ECH // 2    # DoubleRow chunk pairs (8)
SB = 512            # s-block width
NSB = S // SB       # 4 s-blocks
NST = S // 128      # 16 s chunks

SX = 16.0           # fp8 scale on x
SW = 64.0           # fp8 scale on weights
SAO = 16.0          # fp8 scale on attention output
PRJ = SX * SW       # projection psum scale (1024)

dt = mybir.dt
F32 = dt.float32
BF16 = dt.bfloat16
F8 = dt.float8e4
AX = mybir.AluOpType
ACTF = mybir.ActivationFunctionType
DR = mybir.MatmulPerfMode.DoubleRow
F8NP = ml_dtypes.float8_e4m3
BFNP = ml_dtypes.bfloat16

_CACHE = {}


def _build_program():
    nc = bacc.Bacc("TRN2", target_bir_lowering=False, debug=False,
                   num_devices=NCORES)

    xhi_d = nc.dram_tensor("xhi", [128, ECH, S], F8, kind="ExternalInput")
    xlo_d = nc.dram_tensor("xlo", [128, ECH, S], F8, kind="ExternalInput")
    w_d = {}
    for nm in ("wq", "wk", "wv"):
        for hl in ("hi", "lo"):
            w_d[nm + hl] = nc.dram_tensor(nm + hl, [128, ECH, FH], F8,
                                          kind="ExternalInput")
    wohi_d = nc.dram_tensor("wohi", [128, HPC, E], F8, kind="ExternalInput")
    wolo_d = nc.dram_tensor("wolo", [128, HPC, E], F8, kind="ExternalInput")
    bqk_d = nc.dram_tensor("bqk", [128, 4 * HPC], F32, kind="ExternalInput")
    bv_d = nc.dram_tensor("bv_rep", [128, FH], BF16, kind="ExternalInput")
    cos_d = nc.dram_tensor("cos_t", [128, S], BF16, kind="ExternalInput")
    sin_d = nc.dram_tensor("sin_t", [128, S], BF16, kind="ExternalInput")
    cmask_d = nc.dram_tensor("cmask", [128, 4 * SB], BF16,
                             kind="ExternalInput")
    out_d = nc.dram_tensor("out", [S, E], BF16, kind="ExternalOutput")

    inv_sqrt_d = float(1.0 / np.sqrt(D))

    with TileContext(nc) as tc:
        with (
            tc.tile_pool(name="psum", bufs=2, space="PSUM") as psp,
            tc.tile_pool(name="cst", bufs=1) as cst,
            tc.tile_pool(name="wp", bufs=1) as wp,
            tc.tile_pool(name="kv", bufs=1) as kvp,
            tc.tile_pool(name="xp", bufs=2) as xp,
            tc.tile_pool(name="st", bufs=2) as st1,
            tc.tile_pool(name="aop", bufs=2) as aop,
            tc.tile_pool(name="osp", bufs=3) as osp,
        ):
            cm_t = cst.tile([128, 4 * SB], BF16, tag="cm")
            bqk_t = cst.tile([128, 4 * HPC], F32, tag="bqk")
            bv_t = cst.tile([128, FH], BF16, tag="bv")
            cos_t = cst.tile([128, S], BF16, tag="cos")
            sin_t = cst.tile([128, S], BF16, tag="sin")
            wu_t = cst.tile([128, SB], BF16, tag="wu")
            nc.any.memset(wu_t[:], 0.5)

            def load_x(sb):
                ssl = slice(sb * SB, (sb + 1) * SB)
                xh = xp.tile([128, ECH, SB], F8, tag="xh", name="xh")
                xl = xp.tile([128, ECH, SB], F8, tag="xl", name="xl")
                nc.sync.dma_start(out=xh[:], in_=xhi_d[:, :, ssl])
                nc.sync.dma_start(out=xl[:], in_=xlo_d[:, :, ssl])
                return xh, xl

            # startup order: the first q chain needs x_hi + wq_hi first,
            # then x_lo + wq_lo (products 2/3), then rope tables.
            ssl0 = slice(0, SB)
            xh0 = xp.tile([128, ECH, SB], F8, tag="xh", name="xh0")
            xl0 = xp.tile([128, ECH, SB], F8, tag="xl", name="xl0")
            nc.sync.dma_start(out=xh0[:], in_=xhi_d[:, :, ssl0])
            wt = {}
            for nm in ("wq", "wk", "wv"):
                for hl in ("hi", "lo"):
                    wt[nm + hl] = wp.tile([128, ECH, FH], F8, tag=nm + hl,
                                          name=nm + hl)
            nc.sync.dma_start(out=wt["wqhi"][:], in_=w_d["wqhi"][:])
            nc.sync.dma_start(out=bqk_t[:], in_=bqk_d[:])
            nc.sync.dma_start(out=xl0[:], in_=xlo_d[:, :, ssl0])
            nc.sync.dma_start(out=wt["wqlo"][:], in_=w_d["wqlo"][:])
            nc.sync.dma_start(out=cos_t[:], in_=cos_d[:])
            nc.sync.dma_start(out=sin_t[:], in_=sin_d[:])
            xs0 = (xh0, xl0)
            nc.sync.dma_start(out=wt["wkhi"][:], in_=w_d["wkhi"][:])
            nc.sync.dma_start(out=wt["wklo"][:], in_=w_d["wklo"][:])
            for hl in ("hi", "lo"):
                nc.sync.dma_start(out=wt["wv" + hl][:], in_=w_d["wv" + hl][:])
            nc.sync.dma_start(out=bv_t[:], in_=bv_d[:])
            nc.sync.dma_start(out=cm_t[:], in_=cmask_d[:])
            wo_hi = wp.tile([128, HPC, E], F8, tag="wohi")
            wo_lo = wp.tile([128, HPC, E], F8, tag="wolo")
            nc.sync.dma_start(out=wo_hi[:], in_=wohi_d[:])
            nc.sync.dma_start(out=wo_lo[:], in_=wolo_d[:])

            # persistent k (per head, [d, S]) and v ([s128, (chunk, h, d)])
            kh = [kvp.tile([128, S], BF16, tag=f"kh{h}", name=f"kh{h}")
                  for h in range(HPC)]
            vh = kvp.tile([128, NST, FH], BF16, tag="vh")

            # PE warm-up: lifts the clock gate while initial DMAs fill
            # (memset input so no DMA dependency).
            pwarm = psp.tile([128, SB], F32, tag="po", bufs=2, name="pwarm")
            for i in range(16):
                nc.tensor.matmul(pwarm[:], wu_t[:, 0:128], wu_t[:, 0:SB],
                                 start=(i == 0), stop=(i == 15))

            def emit_qk1(sb, h, xs, kinds=("q", "k")):
                """Project+rope q and/or k for head h of s-block sb."""
                xh, xl = xs
                ssl = slice(sb * SB, (sb + 1) * SB)
                fsl = slice(h * 128, (h + 1) * 128)
                qtile = None
                for kind in kinds:
                    whi = wt[("wq" if kind == "q" else "wk") + "hi"]
                    wlo = wt[("wq" if kind == "q" else "wk") + "lo"]
                    ps = psp.tile([128, SB], F32, tag="ps1", bufs=4,
                                  name="psqk")
                    for j in range(NPAIR):
                        jp = slice(2 * j, 2 * j + 2)
                        nc.tensor.matmul(ps[:], whi[:, jp, fsl], xh[:, jp, :],
                                         start=(j == 0), stop=False,
                                         perf_mode=DR)
                    for j in range(NPAIR):
                        jp = slice(2 * j, 2 * j + 2)
                        nc.tensor.matmul(ps[:], whi[:, jp, fsl], xl[:, jp, :],
                                         start=False, stop=False,
                                         perf_mode=DR)
                    for j in range(NPAIR):
                        jp = slice(2 * j, 2 * j + 2)
                        nc.tensor.matmul(ps[:], wlo[:, jp, fsl], xh[:, jp, :],
                                         start=False, stop=(j == NPAIR - 1),
                                         perf_mode=DR)
                    bofs = 0 if kind == "q" else 2 * HPC
                    bias = bqk_t[:, bofs + h:bofs + h + 1]
                    bias_sw = bqk_t[:, bofs + HPC + h:bofs + HPC + h + 1]
                    t1 = st1.tile([128, SB], BF16, tag="t1", bufs=2)
                    nc.vector.scalar_tensor_tensor(
                        out=t1[:], in0=ps[:], scalar=bias,
                        in1=cos_t[:, ssl], op0=AX.add, op1=AX.mult)
                    t2 = st1.tile([128, SB], BF16, tag="t2", bufs=2)
                    nc.vector.scalar_tensor_tensor(
                        out=t2[0:64, :], in0=ps[64:128, :],
                        scalar=bias_sw[0:64], in1=sin_t[0:64, ssl],
                        op0=AX.add, op1=AX.mult)
                    nc.vector.scalar_tensor_tensor(
                        out=t2[64:128, :], in0=ps[0:64, :],
                        scalar=bias_sw[64:128], in1=sin_t[64:128, ssl],
                        op0=AX.add, op1=AX.mult)
                    if kind == "q":
                        dst = st1.tile([128, SB], BF16, tag="qh", bufs=8,
                                       name="qh")
                        qtile = dst
                        dview = dst[:]
                    else:
                        dview = kh[h][:, ssl]
                    nc.gpsimd.tensor_add(dview, t1[:], t2[:])
                return qtile

            def emit_qk(sb, h, xs):
                return emit_qk1(sb, h, xs)

            def emit_qk_prologue(kind, xs):
                """Product-major emission across all 4 heads (startup:
                later products' weights arrive while earlier ones run)."""
                xh, xl = xs
                whi = wt[("wq" if kind == "q" else "wk") + "hi"]
                wlo = wt[("wq" if kind == "q" else "wk") + "lo"]
                pss = [psp.tile([128, SB], F32, tag="ps1", bufs=4,
                                name=f"pspro{kind}{h}") for h in range(HPC)]
                for xt, wtl, first, last in ((xh, whi, True, False),
                                             (xl, whi, False, False),
                                             (xh, wlo, False, True)):
                    for h in range(HPC):
                        fsl = slice(h * 128, (h + 1) * 128)
                        for j in range(NPAIR):
                            jp = slice(2 * j, 2 * j + 2)
                            nc.tensor.matmul(
                                pss[h][:], wtl[:, jp, fsl], xt[:, jp, :],
                                start=(first and j == 0),
                                stop=(last and j == NPAIR - 1), perf_mode=DR)
                out = []
                for h in range(HPC):
                    ps = pss[h]
                    ssl = slice(0, SB)
                    bofs = 0 if kind == "q" else 2 * HPC
                    bias = bqk_t[:, bofs + h:bofs + h + 1]
                    bias_sw = bqk_t[:, bofs + HPC + h:bofs + HPC + h + 1]
                    t1 = st1.tile([128, SB], BF16, tag="t1", bufs=2)
                    nc.vector.scalar_tensor_tensor(
                        out=t1[:], in0=ps[:], scalar=bias,
                        in1=cos_t[:, ssl], op0=AX.add, op1=AX.mult)
                    t2 = st1.tile([128, SB], BF16, tag="t2", bufs=2)
                    nc.vector.scalar_tensor_tensor(
                        out=t2[0:64, :], in0=ps[64:128, :],
                        scalar=bias_sw[0:64], in1=sin_t[0:64, ssl],
                        op0=AX.add, op1=AX.mult)
                    nc.vector.scalar_tensor_tensor(
                        out=t2[64:128, :], in0=ps[0:64, :],
                        scalar=bias_sw[64:128], in1=sin_t[64:128, ssl],
                        op0=AX.add, op1=AX.mult)
                    if kind == "q":
                        dst = st1.tile([128, SB], BF16, tag="qh", bufs=8,
                                       name="qh")
                        out.append(dst)
                        dview = dst[:]
                    else:
                        dview = kh[h][:, ssl]
                    nc.gpsimd.tensor_add(dview, t1[:], t2[:])
                return out

            def emit_v(sb, xs):
                xh, xl = xs
                for ssub in range(SB // 128):
                    scol = slice(ssub * 128, (ssub + 1) * 128)
                    ps = psp.tile([128, FH], F32, tag="ps1", bufs=4,
                                  name="psv")
                    for j in range(NPAIR):
                        jp = slice(2 * j, 2 * j + 2)
                        nc.tensor.matmul(ps[:], xh[:, jp, scol],
                                         wt["wvhi"][:, jp, :],
                                         start=(j == 0), stop=False,
                                         perf_mode=DR)
                    for j in range(NPAIR):
                        jp = slice(2 * j, 2 * j + 2)
                        nc.tensor.matmul(ps[:], xl[:, jp, scol],
                                         wt["wvhi"][:, jp, :],
                                         start=False, stop=False,
                                         perf_mode=DR)
                    for j in range(NPAIR):
                        jp = slice(2 * j, 2 * j + 2)
                        nc.tensor.matmul(ps[:], xh[:, jp, scol],
                                         wt["wvlo"][:, jp, :],
                                         start=False, stop=(j == NPAIR - 1),
                                         perf_mode=DR)
                    # vh = ps/PRJ + bv  (bf16)
                    nc.vector.scalar_tensor_tensor(
                        out=vh[:, sb * 4 + ssub, :], in0=ps[:],
                        scalar=float(1.0 / PRJ), in1=bv_t[:],
                        op0=AX.mult, op1=AX.add)

            def attn_gen(sb, h, qtile, aohi, aolo):
                """Causal attention q-tile sb for head h (bf16); generator
                yielding once per k-chunk so two heads can interleave."""
                nk = (sb + 1) * 4
                po = psp.tile([128, SB], F32, tag="po", bufs=2, name="po")
                dacc = st1.tile([128, SB], F32, tag="dacc", bufs=3)
                pexps = []      # (ki, pexp, q0)
                pending = []
                for ki in range(nk):
                    j = ki - sb * 4
                    # diagonal chunks compute scores from their pair's q0 so
                    # pair ops (dacc) see fully-masked zeros; attn@V skips
                    # the known-zero extension
                    q0 = 128 * (j - (j % 2)) if j > 0 else 0
                    q0v = 128 * j if j > 0 else 0
                    ksl = slice(ki * 128, (ki + 1) * 128)
                    pscore = psp.tile([128, SB], F32, tag="pscore",
                                      bufs=2, name="pscore")
                    nc.tensor.matmul(pscore[:, q0:SB], kh[h][:, ksl],
                                     qtile[:, q0:SB], start=True, stop=True)
                    pexp = st1.tile([128, SB], BF16, tag="pexp", bufs=10)
                    nc.scalar.activation(pexp[:, q0:SB], pscore[:, q0:SB],
                                         ACTF.Exp, scale=inv_sqrt_d)
                    if j >= 0:
                        nc.vector.tensor_mul(
                            pexp[:, q0:SB], pexp[:, q0:SB],
                            cm_t[:, j * SB + q0:(j + 1) * SB])
                    pexps.append((ki, pexp, q0))
                    del q0
                    if ki % 2 == 1:
                        _, pa, pq0 = pexps[ki - 1]
                        if ki == 1:
                            nc.vector.tensor_add(dacc[:, pq0:SB],
                                                 pa[:, pq0:SB],
                                                 pexp[:, pq0:SB])
                        else:
                            sp = st1.tile([128, SB], BF16, tag="spair",
                                          bufs=3)
                            nc.vector.tensor_add(sp[:, pq0:SB],
                                                 pa[:, pq0:SB],
                                                 pexp[:, pq0:SB])
                            nc.vector.tensor_add(dacc[:, pq0:SB],
                                                 dacc[:, pq0:SB],
                                                 sp[:, pq0:SB])
                    pending.append((ki, pexp, q0v))
                    if len(pending) > 3:
                        k0, px, pq = pending.pop(0)
                        nc.tensor.matmul(
                            po[:, pq:SB],
                            vh[:, k0, h * 128:(h + 1) * 128],
                            px[:, pq:SB], start=(k0 == 0), stop=False)
                    yield
                while pending:
                    k0, px, pq = pending.pop(0)
                    last = not pending
                    nc.tensor.matmul(po[:, pq:SB],
                                     vh[:, k0, h * 128:(h + 1) * 128],
                                     px[:, pq:SB], start=(k0 == 0), stop=last)
                dred = st1.tile([128, SB], F32, tag="dred", bufs=2)
                nc.gpsimd.partition_all_reduce(
                    out_ap=dred[:], in_ap=dacc[:], channels=128,
                    reduce_op=__import__("concourse.bass_isa",
                                         fromlist=["ReduceOp"]).ReduceOp.add)
                rec = st1.tile([128, SB], F32, tag="rec", bufs=3)
                nc.vector.reciprocal(rec[:], dred[:])
                t = st1.tile([128, SB], BF16, tag="taot", bufs=3)
                nc.vector.scalar_tensor_tensor(
                    out=t[:], in0=po[:], scalar=SAO, in1=rec[:],
                    op0=AX.mult, op1=AX.mult)
                nc.scalar.copy(aohi[:, h, :], t[:])
                nc.vector.tensor_sub(aolo[:, h, :], t[:], aohi[:, h, :])

            def emit_outproj_piece(sb, sti, aohi, aolo):
                """Out-projection for one 128-row s-chunk of s-block sb.
                Head-pair-major chain order so the first half only depends
                on heads 0/1."""
                stsl = slice(sti * 128, (sti + 1) * 128)
                osb = osp.tile([128, E], BF16, tag="osb", name="osb")
                for gt in range(E // 512):
                    gsl = slice(gt * 512, (gt + 1) * 512)
                    psO = psp.tile([128, 512], F32, tag="ps1", bufs=4,
                                   name="psO")
                    n = 0
                    for j in range(HPC // 2):
                        jp = slice(2 * j, 2 * j + 2)
                        for lhs, rhs in ((aohi, wo_hi), (aolo, wo_hi),
                                         (aohi, wo_lo)):
                            nc.tensor.matmul(psO[:], lhs[:, jp, stsl],
                                             rhs[:, jp, gsl],
                                             start=(n == 0), stop=(n == 5),
                                             perf_mode=DR)
                            n += 1
                    nc.scalar.activation(osb[:, gsl], psO[:], ACTF.Copy,
                                         scale=float(1.0 / (SAO * SW)))
                    row0 = sb * SB + sti * 128
                    nc.sync.dma_start(out=out_d[row0:row0 + 128, gsl],
                                      in_=osb[:, gsl])

            def drive_pair(*gens):
                """Round-robin attention generators."""
                live = list(gens)
                while live:
                    for g in list(live):
                        try:
                            next(g)
                        except StopIteration:
                            live.remove(g)

            # ---- stage loop: attn(sb) head-pairs interleaved, plus
            # proj(sb+1) and out-projection pieces of sb-1 ----
            q_cur = emit_qk_prologue("q", xs0)
            emit_qk_prologue("k", xs0)
            emit_v(0, xs0)
            prev = None     # (sb-1, aohi, aolo) with outproj still pending
            for sb in range(NSB):
                nxt = sb + 1
                if nxt < NSB:
                    xs_n = load_x(nxt)
                aohi = aop.tile([128, HPC, SB], F8, tag="aohi", name="aohi")
                aolo = aop.tile([128, HPC, SB], F8, tag="aolo", name="aolo")
                q_next = []
                if sb < NSB - 1:
                    for h in range(HPC):
                        drive_pair(attn_gen(sb, h, q_cur[h], aohi, aolo))
                        if nxt < NSB:
                            q_next.append(emit_qk(nxt, h, xs_n))
                        if prev is not None:
                            emit_outproj_piece(prev[0], h, prev[1], prev[2])
                else:
                    # final stage: no next-stage projections to hide the
                    # exp->mask->po latency, so interleave head pairs and
                    # keep the previous block's out-proj pieces for filler
                    drive_pair(attn_gen(sb, 0, q_cur[0], aohi, aolo),
                               attn_gen(sb, 1, q_cur[1], aohi, aolo))
                    emit_outproj_piece(prev[0], 0, prev[1], prev[2])
                    emit_outproj_piece(prev[0], 1, prev[1], prev[2])
                    drive_pair(attn_gen(sb, 2, q_cur[2], aohi, aolo),
                               attn_gen(sb, 3, q_cur[3], aohi, aolo))
                    emit_outproj_piece(prev[0], 2, prev[1], prev[2])
                    emit_outproj_piece(prev[0], 3, prev[1], prev[2])
                if nxt < NSB:
                    emit_v(nxt, xs_n)
                prev = (sb, aohi, aolo)
                q_cur = q_next
            for sti in range(SB // 128):
                emit_outproj_piece(prev[0], sti, prev[1], prev[2])

    nc.compile()
    return nc


def _host_constants():
    """RoPE cos/sin tables (evens-first, pre-descaled) and causal masks."""
    i = np.arange(64, dtype=np.float64)
    freqs = np.power(10000.0, -2.0 * i / D)
    pos = np.arange(S, dtype=np.float64)
    ang = pos[None, :] * freqs[:, None]              # [64, S]
    cos = np.cos(ang)
    sin = np.sin(ang)
    cos_t = (np.concatenate([cos, cos], axis=0) / PRJ).astype(BFNP)
    sin_t = (np.concatenate([-sin, sin], axis=0) / PRJ).astype(BFNP)
    r = np.arange(128)[:, None]
    c = np.arange(SB)[None, :]
    masks = [(128 * j + r <= c).astype(np.float32) for j in range(4)]
    cmask = np.concatenate(masks, axis=1).astype(BFNP)
    return cos_t, sin_t, cmask


def _split8(t, s):
    hi = (s * t).astype(F8NP)
    lo = (s * t - hi.astype(np.float32)).astype(F8NP)
    return hi, lo


def _chunked(t, nch):
    """[nch*128, N] f8 -> [128, nch, N]"""
    n = t.shape[1]
    return np.ascontiguousarray(
        t.reshape(nch, 128, n).transpose(1, 0, 2))


def kernel(x, Wq, bq, Wk, bk, Wv, bv, Wo, bo):
    x = np.asarray(x, dtype=np.float32)
    Wq = np.asarray(Wq, dtype=np.float32)
    bq = np.asarray(bq, dtype=np.float32)
    Wk = np.asarray(Wk, dtype=np.float32)
    bk = np.asarray(bk, dtype=np.float32)
    Wv = np.asarray(Wv, dtype=np.float32)
    bv = np.asarray(bv, dtype=np.float32)
    Wo = np.asarray(Wo, dtype=np.float32)
    bo = np.asarray(bo, dtype=np.float32)

    if "nc" not in _CACHE:
        _CACHE["nc"] = _build_program()
        _CACHE["consts"] = _host_constants()
    nc = _CACHE["nc"]
    cos_t, sin_t, cmask = _CACHE["consts"]

    perm = np.concatenate([np.arange(0, D, 2), np.arange(1, D, 2)])
    sw64 = np.concatenate([np.arange(64, 128), np.arange(0, 64)])

    xsplit = []
    for b in range(B):
        xT = np.ascontiguousarray(x[b].T)
        xh, xl = _split8(xT, SX)
        xsplit.append((_chunked(xh, ECH), _chunked(xl, ECH)))

    in_maps = []
    for c in range(NCORES):
        b, hg = divmod(c, GROUPS)
        rows = slice(hg * FH, (hg + 1) * FH)
        Wq_s = Wq[rows].reshape(HPC, D, E)[:, perm, :].reshape(FH, E)
        Wk_s = Wk[rows].reshape(HPC, D, E)[:, perm, :].reshape(FH, E)
        bq_s = bq[rows].reshape(HPC, D)[:, perm]     # [HPC, 128]
        bk_s = bk[rows].reshape(HPC, D)[:, perm]
        bqk_t = PRJ * np.concatenate(
            [bq_s, bq_s[:, sw64], bk_s, bk_s[:, sw64]],
            axis=0).T.astype(np.float32)
        bqk_t = np.ascontiguousarray(bqk_t)          # [128, 4*HPC]

        wqh, wql = _split8(np.ascontiguousarray(Wq_s.T), SW)
        wkh, wkl = _split8(np.ascontiguousarray(Wk_s.T), SW)
        wvh, wvl = _split8(np.ascontiguousarray(Wv[rows].T), SW)
        woh, wol = _split8(np.ascontiguousarray(Wo[:, rows].T), SW)

        in_maps.append({
            "xhi": xsplit[b][0],
            "xlo": xsplit[b][1],
            "wqhi": _chunked(wqh, ECH), "wqlo": _chunked(wql, ECH),
            "wkhi": _chunked(wkh, ECH), "wklo": _chunked(wkl, ECH),
            "wvhi": _chunked(wvh, ECH), "wvlo": _chunked(wvl, ECH),
            "wohi": _chunked(woh, HPC), "wolo": _chunked(wol, HPC),
            "bqk": bqk_t,
            "bv_rep": np.ascontiguousarray(
                np.broadcast_to(bv[rows], (128, FH))).astype(BFNP),
            "cos_t": cos_t,
            "sin_t": sin_t,
            "cmask": cmask,
        })

    res = run_bass_kernel_spmd(nc, in_maps, list(range(NCORES)))
    outs = [res.results[c]["out"] for c in range(NCORES)]

    result = np.empty((B, S, E), dtype=np.float32)
    for b in range(B):
        acc = outs[GROUPS * b].astype(np.float32)
        for g in range(1, GROUPS):
            acc = acc + outs[GROUPS * b + g].astype(np.float32)
        result[b] = acc + bo[None, :]
    return result


# revision 44
# speedup vs baseline: 1.0854x; 1.0137x over previous
"""Multi-head self-attention with RoPE on 8 TRN2 NeuronCores.

Sharding: core c = (b, hg): b = c // 4 (data parallel over batch),
hg = c % 4 (tensor parallel over head groups of 4 heads = 512 features).
Each core computes q/k/v projections for its 4 heads, RoPE, causal
attention, and a partial out-projection [S, E] in bf16; the host sums
the 4 partials per batch and adds bo.

Performance scheme (single pass over all 4 heads):
- q/k/v and out projections run as fp8e4m3 DoubleRow matmuls (0.5
  cycles/row, 2x128 contraction per instruction) with a 3-product
  hi/lo residual split (x_hi@W_hi + x_lo@W_hi + x_hi@W_lo) that keeps
  quantization error at the ~1e-3 level: 0.75 cycles per fp32r-row
  equivalent. x and W splits are precomputed on the host; the
  attention-output split is computed on device (ACT cast + DVE sub).
- Attention (scores, attn@V) runs in bf16 (1 cycle/row, exact f32
  accumulation in PSUM). RoPE runs as 3 DVE scalar_tensor_tensor ops
  per tensor (full-width cos term + two half-partition swapped sin
  terms) with the 1/1024 projection descale folded into the bf16
  cos/sin tables, combining on the Pool engine.
- The softmax denominator accumulates in f32 from bf16 chunk-pair sums
  (DVE), is partition-reduced on Pool, and reciprocal'd on DVE. Causal
  masks multiply on Pool. exp runs on ACT writing bf16.
- Diagonal k-chunks compute only from the covering pair start so
  chunk-pair ops (dacc) see fully-masked zeros in the extension.
"""

import sys

if "/opt/trn_rl_repo" not in sys.path:
    sys.path.insert(0, "/opt/trn_rl_repo")

import numpy as np
import ml_dtypes

import concourse.bass as bass  # noqa: F401
import concourse.mybir as mybir
from concourse import bacc
from concourse.tile import TileContext
from concourse.bass_utils import run_bass_kernel_spmd

B, S, E, H, D = 2, 2048, 2048, 16, 128
NCORES = 8
GROUPS = 4          # head groups (tensor parallel)
HPC = H // GROUPS   # heads per core (4)
FH = HPC * D        # features per core (512)
ECH = E // 128      # contraction chunks (16)
NPAIR = ECH // 2    # DoubleRow chunk pairs (8)
SB = 512            # s-block width
NSB = S // SB       # 4 s-blocks
NST = S // 128      # 16 s chunks

SX = 16.0           # fp8 scale on x
SW = 64.0           # fp8 scale on weights
SAO = 16.0          # fp8 scale on attention output
PRJ = SX * SW       # projection psum scale (1024)

dt = mybir.dt
F32 = dt.float32
BF16 = dt.bfloat16
F8 = dt.float8e4
AX = mybir.AluOpType
ACTF = mybir.ActivationFunctionType
DR = mybir.MatmulPerfMode.DoubleRow
F8NP = ml_dtypes.float8_e4m3
BFNP = ml_dtypes.bfloat16

_CACHE = {}


def _build_program():
    nc = bacc.Bacc("TRN2", target_bir_lowering=False, debug=False,
                   num_devices=NCORES)

    xhi_d = nc.dram_tensor("xhi", [128, ECH, S], F8, kind="ExternalInput")
    xlo_d = nc.dram_tensor("xlo", [128, ECH, S], F8, kind="ExternalInput")
    w_d = {}
    for nm in ("wq", "wk", "wv"):
        for hl in ("hi", "lo"):
            w_d[nm + hl] = nc.dram_tensor(nm + hl, [128, ECH, FH], F8,
                                          kind="ExternalInput")
    wohi_d = nc.dram_tensor("wohi", [128, HPC, E], F8, kind="ExternalInput")
    wolo_d = nc.dram_tensor("wolo", [128, HPC, E], F8, kind="ExternalInput")
    bqk_d = nc.dram_tensor("bqk", [128, 4 * HPC], F32, kind="ExternalInput")
    bv_d = nc.dram_tensor("bv_rep", [128, FH], BF16, kind="ExternalInput")
    cos_d = nc.dram_tensor("cos_t", [128, S], BF16, kind="ExternalInput")
    sin_d = nc.dram_tensor("sin_t", [128, S], BF16, kind="ExternalInput")
    cmask_d = nc.dram_tensor("cmask", [128, 4 * SB], BF16,
                             kind="ExternalInput")
    out_d = nc.dram_tensor("out", [S, E], BF16, kind="ExternalOutput")

    inv_sqrt_d = float(1.0 / np.sqrt(D))

    with TileContext(nc) as tc:
        with (
            tc.tile_pool(name="psum", bufs=2, space="PSUM") as psp,
            tc.tile_pool(name="cst", bufs=1) as cst,
            tc.tile_pool(name="wp", bufs=1) as wp,
            tc.tile_pool(name="kv", bufs=1) as kvp,
            tc.tile_pool(name="xp", bufs=2) as xp,
            tc.tile_pool(name="st", bufs=2) as st1,
            tc.tile_pool(name="aop", bufs=2) as aop,
            tc.tile_pool(name="osp", bufs=3) as osp,
        ):
            cm_t = cst.tile([128, 4 * SB], BF16, tag="cm")
            bqk_t = cst.tile([128, 4 * HPC], F32, tag="bqk")
            bv_t = cst.tile([128, FH], BF16, tag="bv")
            cos_t = cst.tile([128, S], BF16, tag="cos")
            sin_t = cst.tile([128, S], BF16, tag="sin")
            wu_t = cst.tile([128, SB], BF16, tag="wu")
            cos0 = cst.tile([128, SB], BF16, tag="cos0")
            sin0 = cst.tile([128, SB], BF16, tag="sin0")
            nc.any.memset(wu_t[:], 0.5)

            def load_x(sb):
                ssl = slice(sb * SB, (sb + 1) * SB)
                xh = xp.tile([128, ECH, SB], F8, tag="xh", name="xh")
                xl = xp.tile([128, ECH, SB], F8, tag="xl", name="xl")
                nc.sync.dma_start(out=xh[:], in_=xhi_d[:, :, ssl])
                nc.sync.dma_start(out=xl[:], in_=xlo_d[:, :, ssl])
                return xh, xl

            # startup order: the first q chain needs x_hi + wq_hi first,
            # then x_lo + wq_lo (products 2/3), then rope tables.
            ssl0 = slice(0, SB)
            xh0 = xp.tile([128, ECH, SB], F8, tag="xh", name="xh0")
            xl0 = xp.tile([128, ECH, SB], F8, tag="xl", name="xl0")
            nc.sync.dma_start(out=xh0[:], in_=xhi_d[:, :, ssl0])
            wt = {}
            for nm in ("wq", "wk", "wv"):
                for hl in ("hi", "lo"):
                    wt[nm + hl] = wp.tile([128, ECH, FH], F8, tag=nm + hl,
                                          name=nm + hl)
            nc.sync.dma_start(out=wt["wqhi"][:], in_=w_d["wqhi"][:])
            nc.sync.dma_start(out=bqk_t[:], in_=bqk_d[:])
            nc.sync.dma_start(out=cos0[:], in_=cos_d[:, 0:SB])
            nc.sync.dma_start(out=sin0[:], in_=sin_d[:, 0:SB])
            nc.sync.dma_start(out=xl0[:], in_=xlo_d[:, :, ssl0])
            nc.sync.dma_start(out=wt["wqlo"][:], in_=w_d["wqlo"][:])
            xs0 = (xh0, xl0)
            nc.sync.dma_start(out=wt["wkhi"][:], in_=w_d["wkhi"][:])
            nc.sync.dma_start(out=wt["wklo"][:], in_=w_d["wklo"][:])
            nc.sync.dma_start(out=cos_t[:], in_=cos_d[:])
            nc.sync.dma_start(out=sin_t[:], in_=sin_d[:])
            for hl in ("hi", "lo"):
                nc.sync.dma_start(out=wt["wv" + hl][:], in_=w_d["wv" + hl][:])
            nc.sync.dma_start(out=bv_t[:], in_=bv_d[:])
            nc.sync.dma_start(out=cm_t[:], in_=cmask_d[:])
            wo_hi = wp.tile([128, HPC, E], F8, tag="wohi")
            wo_lo = wp.tile([128, HPC, E], F8, tag="wolo")
            nc.sync.dma_start(out=wo_hi[:], in_=wohi_d[:])
            nc.sync.dma_start(out=wo_lo[:], in_=wolo_d[:])

            # persistent k (per head, [d, S]) and v ([s128, (chunk, h, d)])
            kh = [kvp.tile([128, S], BF16, tag=f"kh{h}", name=f"kh{h}")
                  for h in range(HPC)]
            vh = kvp.tile([128, NST, FH], BF16, tag="vh")

            # PE warm-up: lifts the clock gate while initial DMAs fill
            # (memset input so no DMA dependency).
            pwarm = psp.tile([128, SB], F32, tag="po", bufs=2, name="pwarm")
            for i in range(16):
                nc.tensor.matmul(pwarm[:], wu_t[:, 0:128], wu_t[:, 0:SB],
                                 start=(i == 0), stop=(i == 15))

            def emit_qk1(sb, h, xs, kinds=("q", "k")):
                """Project+rope q and/or k for head h of s-block sb."""
                xh, xl = xs
                ssl = slice(sb * SB, (sb + 1) * SB)
                fsl = slice(h * 128, (h + 1) * 128)
                qtile = None
                for kind in kinds:
                    whi = wt[("wq" if kind == "q" else "wk") + "hi"]
                    wlo = wt[("wq" if kind == "q" else "wk") + "lo"]
                    ps = psp.tile([128, SB], F32, tag="ps1", bufs=4,
                                  name="psqk")
                    for j in range(NPAIR):
                        jp = slice(2 * j, 2 * j + 2)
                        nc.tensor.matmul(ps[:], whi[:, jp, fsl], xh[:, jp, :],
                                         start=(j == 0), stop=False,
                                         perf_mode=DR)
                    for j in range(NPAIR):
                        jp = slice(2 * j, 2 * j + 2)
                        nc.tensor.matmul(ps[:], whi[:, jp, fsl], xl[:, jp, :],
                                         start=False, stop=False,
                                         perf_mode=DR)
                    for j in range(NPAIR):
                        jp = slice(2 * j, 2 * j + 2)
                        nc.tensor.matmul(ps[:], wlo[:, jp, fsl], xh[:, jp, :],
                                         start=False, stop=(j == NPAIR - 1),
                                         perf_mode=DR)
                    bofs = 0 if kind == "q" else 2 * HPC
                    bias = bqk_t[:, bofs + h:bofs + h + 1]
                    bias_sw = bqk_t[:, bofs + HPC + h:bofs + HPC + h + 1]
                    t1 = st1.tile([128, SB], BF16, tag="t1", bufs=2)
                    nc.vector.scalar_tensor_tensor(
                        out=t1[:], in0=ps[:], scalar=bias,
                        in1=cos_t[:, ssl], op0=AX.add, op1=AX.mult)
                    t2 = st1.tile([128, SB], BF16, tag="t2", bufs=2)
                    nc.vector.scalar_tensor_tensor(
                        out=t2[0:64, :], in0=ps[64:128, :],
                        scalar=bias_sw[0:64], in1=sin_t[0:64, ssl],
                        op0=AX.add, op1=AX.mult)
                    nc.vector.scalar_tensor_tensor(
                        out=t2[64:128, :], in0=ps[0:64, :],
                        scalar=bias_sw[64:128], in1=sin_t[64:128, ssl],
                        op0=AX.add, op1=AX.mult)
                    if kind == "q":
                        dst = st1.tile([128, SB], BF16, tag="qh", bufs=8,
                                       name="qh")
                        qtile = dst
                        dview = dst[:]
                    else:
                        dview = kh[h][:, ssl]
                    nc.gpsimd.tensor_add(dview, t1[:], t2[:])
                return qtile

            def emit_qk(sb, h, xs):
                return emit_qk1(sb, h, xs)

            def emit_qk_prologue(kind, xs):
                """Product-major emission across all 4 heads (startup:
                later products' weights arrive while earlier ones run)."""
                xh, xl = xs
                whi = wt[("wq" if kind == "q" else "wk") + "hi"]
                wlo = wt[("wq" if kind == "q" else "wk") + "lo"]
                if kind == "q":
                    pss = [psp.tile([128, SB], F32, tag="ps1", bufs=4,
                                    name=f"psproq{h}") for h in range(HPC)]
                else:
                    # attention tags are idle during the prologue; using
                    # them decouples k chains from q psum-bank releases
                    pss = [psp.tile([128, SB], F32, tag="pscore", bufs=2,
                                    name="psprok0"),
                           psp.tile([128, SB], F32, tag="pscore", bufs=2,
                                    name="psprok1"),
                           psp.tile([128, SB], F32, tag="po", bufs=2,
                                    name="psprok2"),
                           psp.tile([128, SB], F32, tag="po", bufs=2,
                                    name="psprok3")]
                for xt, wtl, first, last in ((xh, whi, True, False),
                                             (xl, whi, False, False),
                                             (xh, wlo, False, True)):
                    for h in range(HPC):
                        fsl = slice(h * 128, (h + 1) * 128)
                        for j in range(NPAIR):
                            jp = slice(2 * j, 2 * j + 2)
                            nc.tensor.matmul(
                                pss[h][:], wtl[:, jp, fsl], xt[:, jp, :],
                                start=(first and j == 0),
                                stop=(last and j == NPAIR - 1), perf_mode=DR)
                out = []
                for h in range(HPC):
                    ps = pss[h]
                    bofs = 0 if kind == "q" else 2 * HPC
                    bias = bqk_t[:, bofs + h:bofs + h + 1]
                    bias_sw = bqk_t[:, bofs + HPC + h:bofs + HPC + h + 1]
                    t1 = st1.tile([128, SB], BF16, tag="t1", bufs=2)
                    nc.vector.scalar_tensor_tensor(
                        out=t1[:], in0=ps[:], scalar=bias,
                        in1=cos0[:], op0=AX.add, op1=AX.mult)
                    t2 = st1.tile([128, SB], BF16, tag="t2", bufs=2)
                    nc.vector.scalar_tensor_tensor(
                        out=t2[0:64, :], in0=ps[64:128, :],
                        scalar=bias_sw[0:64], in1=sin0[0:64, :],
                        op0=AX.add, op1=AX.mult)
                    nc.vector.scalar_tensor_tensor(
                        out=t2[64:128, :], in0=ps[0:64, :],
                        scalar=bias_sw[64:128], in1=sin0[64:128, :],
                        op0=AX.add, op1=AX.mult)
                    if kind == "q":
                        dst = st1.tile([128, SB], BF16, tag="qh", bufs=8,
                                       name="qh")
                        out.append(dst)
                        dview = dst[:]
                    else:
                        dview = kh[h][:, 0:SB]
                    nc.gpsimd.tensor_add(dview, t1[:], t2[:])
                return out

            def emit_v(sb, xs):
                xh, xl = xs
                for ssub in range(SB // 128):
                    scol = slice(ssub * 128, (ssub + 1) * 128)
                    ps = psp.tile([128, FH], F32, tag="ps1", bufs=4,
                                  name="psv")
                    for j in range(NPAIR):
                        jp = slice(2 * j, 2 * j + 2)
                        nc.tensor.matmul(ps[:], xh[:, jp, scol],
                                         wt["wvhi"][:, jp, :],
                                         start=(j == 0), stop=False,
                                         perf_mode=DR)
                    for j in range(NPAIR):
                        jp = slice(2 * j, 2 * j + 2)
                        nc.tensor.matmul(ps[:], xl[:, jp, scol],
                                         wt["wvhi"][:, jp, :],
                                         start=False, stop=False,
                                         perf_mode=DR)
                    for j in range(NPAIR):
                        jp = slice(2 * j, 2 * j + 2)
                        nc.tensor.matmul(ps[:], xh[:, jp, scol],
                                         wt["wvlo"][:, jp, :],
                                         start=False, stop=(j == NPAIR - 1),
                                         perf_mode=DR)
                    # vh = ps/PRJ + bv  (bf16)
                    nc.vector.scalar_tensor_tensor(
                        out=vh[:, sb * 4 + ssub, :], in0=ps[:],
                        scalar=float(1.0 / PRJ), in1=bv_t[:],
                        op0=AX.mult, op1=AX.add)

            def attn_gen(sb, h, qtile, aohi, aolo):
                """Causal attention q-tile sb for head h (bf16); generator
                yielding once per k-chunk so two heads can interleave."""
                nk = (sb + 1) * 4
                po = psp.tile([128, SB], F32, tag="po", bufs=2, name="po")
                dacc = st1.tile([128, SB], F32, tag="dacc", bufs=3)
                pexps = []      # (ki, pexp, q0)
                pending = []
                for ki in range(nk):
                    j = ki - sb * 4
                    # diagonal chunks compute scores from their pair's q0 so
                    # pair ops (dacc) see fully-masked zeros; attn@V skips
                    # the known-zero extension
                    q0 = 128 * (j - (j % 2)) if j > 0 else 0
                    q0v = 128 * j if j > 0 else 0
                    ksl = slice(ki * 128, (ki + 1) * 128)
                    pscore = psp.tile([128, SB], F32, tag="pscore",
                                      bufs=2, name="pscore")
                    nc.tensor.matmul(pscore[:, q0:SB], kh[h][:, ksl],
                                     qtile[:, q0:SB], start=True, stop=True)
                    pexp = st1.tile([128, SB], BF16, tag="pexp", bufs=9)
                    nc.scalar.activation(pexp[:, q0:SB], pscore[:, q0:SB],
                                         ACTF.Exp, scale=inv_sqrt_d)
                    if j >= 0:
                        nc.vector.tensor_mul(
                            pexp[:, q0:SB], pexp[:, q0:SB],
                            cm_t[:, j * SB + q0:(j + 1) * SB])
                    pexps.append((ki, pexp, q0))
                    del q0
                    if ki % 2 == 1:
                        _, pa, pq0 = pexps[ki - 1]
                        if ki == 1:
                            nc.vector.tensor_add(dacc[:, pq0:SB],
                                                 pa[:, pq0:SB],
                                                 pexp[:, pq0:SB])
                        else:
                            sp = st1.tile([128, SB], BF16, tag="spair",
                                          bufs=3)
                            nc.vector.tensor_add(sp[:, pq0:SB],
                                                 pa[:, pq0:SB],
                                                 pexp[:, pq0:SB])
                            nc.vector.tensor_add(dacc[:, pq0:SB],
                                                 dacc[:, pq0:SB],
                                                 sp[:, pq0:SB])
                    pending.append((ki, pexp, q0v))
                    if len(pending) > 3:
                        k0, px, pq = pending.pop(0)
                        nc.tensor.matmul(
                            po[:, pq:SB],
                            vh[:, k0, h * 128:(h + 1) * 128],
                            px[:, pq:SB], start=(k0 == 0), stop=False)
                    yield
                while pending:
                    k0, px, pq = pending.pop(0)
                    last = not pending
                    nc.tensor.matmul(po[:, pq:SB],
                                     vh[:, k0, h * 128:(h + 1) * 128],
                                     px[:, pq:SB], start=(k0 == 0), stop=last)
                dred = st1.tile([128, SB], F32, tag="dred", bufs=2)
                nc.gpsimd.partition_all_reduce(
                    out_ap=dred[:], in_ap=dacc[:], channels=128,
                    reduce_op=__import__("concourse.bass_isa",
                                         fromlist=["ReduceOp"]).ReduceOp.add)
                rec = st1.tile([128, SB], F32, tag="rec", bufs=3)
                nc.vector.reciprocal(rec[:], dred[:])
                nc.vector.scalar_tensor_tensor(
                    out=aohi[:, h, :], in0=po[:], scalar=SAO, in1=rec[:],
                    op0=AX.mult, op1=AX.mult)
                t = st1.tile([128, SB], BF16, tag="taot", bufs=3)
                nc.vector.scalar_tensor_tensor(
                    out=t[:], in0=po[:], scalar=SAO, in1=rec[:],
                    op0=AX.mult, op1=AX.mult)
                nc.vector.tensor_sub(aolo[:, h, :], t[:], aohi[:, h, :])

            def emit_outproj_piece(sb, sti, aohi, aolo, last=False):
                """Out-projection for one 128-row s-chunk of s-block sb.
                Chain order puts ao_lo-dependent products last. For the
                final pieces, copies alternate ACT/DVE and the output goes
                out as one batched DMA to shorten the drain tail."""
                stsl = slice(sti * 128, (sti + 1) * 128)
                row0 = sb * SB + sti * 128
                osb = osp.tile([128, E], BF16, tag="osb", name="osb")
                for gt in range(E // 512):
                    gsl = slice(gt * 512, (gt + 1) * 512)
                    psO = psp.tile([128, 512], F32, tag="ps1", bufs=4,
                                   name="psO")
                    n = 0
                    for j in range(HPC // 2):
                        jp = slice(2 * j, 2 * j + 2)
                        for lhs, rhs in ((aohi, wo_hi), (aohi, wo_lo),
                                         (aolo, wo_hi)):
                            nc.tensor.matmul(psO[:], lhs[:, jp, stsl],
                                             rhs[:, jp, gsl],
                                             start=(n == 0), stop=(n == 5),
                                             perf_mode=DR)
                            n += 1
                    if last and gt % 2 == 0:
                        nc.vector.tensor_scalar_mul(
                            osb[:, gsl], psO[:], float(1.0 / (SAO * SW)))
                    else:
                        nc.scalar.activation(osb[:, gsl], psO[:], ACTF.Copy,
                                             scale=float(1.0 / (SAO * SW)))
                    if not last:
                        nc.sync.dma_start(out=out_d[row0:row0 + 128, gsl],
                                          in_=osb[:, gsl])
                if last:
                    nc.sync.dma_start(out=out_d[row0:row0 + 128, :],
                                      in_=osb[:])

            def drive_pair(*gens):
                """Round-robin attention generators."""
                live = list(gens)
                while live:
                    for g in list(live):
                        try:
                            next(g)
                        except StopIteration:
                            live.remove(g)

            # ---- stage loop: attn(sb) head-pairs interleaved, plus
            # proj(sb+1) and out-projection pieces of sb-1 ----
            q_cur = emit_qk_prologue("q", xs0)
            emit_qk_prologue("k", xs0)
            emit_v(0, xs0)
            prev = None     # (sb-1, aohi, aolo) with outproj still pending
            for sb in range(NSB):
                nxt = sb + 1
                if nxt < NSB:
                    xs_n = load_x(nxt)
                aohi = aop.tile([128, HPC, SB], F8, tag="aohi", name="aohi")
                aolo = aop.tile([128, HPC, SB], F8, tag="aolo", name="aolo")
                q_next = []
                if sb < NSB - 1:
                    for h in range(HPC):
                        drive_pair(attn_gen(sb, h, q_cur[h], aohi, aolo))
                        if nxt < NSB:
                            q_next.append(emit_qk(nxt, h, xs_n))
                        if prev is not None:
                            emit_outproj_piece(prev[0], h, prev[1], prev[2])
                else:
                    # final stage: no next-stage projections to hide the
                    # exp->mask->po latency, so interleave head pairs and
                    # keep the previous block's out-proj pieces for filler
                    drive_pair(attn_gen(sb, 0, q_cur[0], aohi, aolo),
                               attn_gen(sb, 1, q_cur[1], aohi, aolo))
                    emit_outproj_piece(prev[0], 0, prev[1], prev[2])
                    emit_outproj_piece(prev[0], 1, prev[1], prev[2])
                    drive_pair(attn_gen(sb, 2, q_cur[2], aohi, aolo),
                               attn_gen(sb, 3, q_cur[3], aohi, aolo))
                    emit_outproj_piece(prev[0], 2, prev[1], prev[2])
                    emit_outproj_piece(prev[0], 3, prev[1], prev[2])
                if nxt < NSB:
                    emit_v(nxt, xs_n)
                prev = (sb, aohi, aolo)
                q_cur = q_next
            for sti in range(SB // 128):
                emit_outproj_piece(prev[0], sti, prev[1], prev[2],
                                   last=True)

    nc.compile()
    return nc


def _host_constants():
    """RoPE cos/sin tables (evens-first, pre-descaled) and causal masks."""
    i = np.arange(64, dtype=np.float64)
    freqs = np.power(10000.0, -2.0 * i / D)
    pos = np.arange(S, dtype=np.float64)
    ang = pos[None, :] * freqs[:, None]              # [64, S]
    cos = np.cos(ang)
    sin = np.sin(ang)
    cos_t = (np.concatenate([cos, cos], axis=0) / PRJ).astype(BFNP)
    sin_t = (np.concatenate([-sin, sin], axis=0) / PRJ).astype(BFNP)
    r = np.arange(128)[:, None]
    c = np.arange(SB)[None, :]
    masks = [(128 * j + r <= c).astype(np.float32) for j in range(4)]
    cmask = np.concatenate(masks, axis=1).astype(BFNP)
    return cos_t, sin_t, cmask


def _split8(t, s):
    hi = (s * t).astype(F8NP)
    lo = (s * t - hi.astype(np.float32)).astype(F8NP)
    return hi, lo


def _chunked(t, nch):
    """[nch*128, N] f8 -> [128, nch, N]"""
    n = t.shape[1]
    return np.ascontiguousarray(
        t.reshape(nch, 128, n).transpose(1, 0, 2))


def kernel(x, Wq, bq, Wk, bk, Wv, bv, Wo, bo):
    x = np.asarray(x, dtype=np.float32)
    Wq = np.asarray(Wq, dtype=np.float32)
    bq = np.asarray(bq, dtype=np.float32)
    Wk = np.asarray(Wk, dtype=np.float32)
    bk = np.asarray(bk, dtype=np.float32)
    Wv = np.asarray(Wv, dtype=np.float32)
    bv = np.asarray(bv, dtype=np.float32)
    Wo = np.asarray(Wo, dtype=np.float32)
    bo = np.asarray(bo, dtype=np.float32)

    if "nc" not in _CACHE:
        _CACHE["nc"] = _build_program()
        _CACHE["consts"] = _host_constants()
    nc = _CACHE["nc"]
    cos_t, sin_t, cmask = _CACHE["consts"]

    perm = np.concatenate([np.arange(0, D, 2), np.arange(1, D, 2)])
    sw64 = np.concatenate([np.arange(64, 128), np.arange(0, 64)])

    xsplit = []
    for b in range(B):
        xT = np.ascontiguousarray(x[b].T)
        xh, xl = _split8(xT, SX)
        xsplit.append((_chunked(xh, ECH), _chunked(xl, ECH)))

    in_maps = []
    for c in range(NCORES):
        b, hg = divmod(c, GROUPS)
        rows = slice(hg * FH, (hg + 1) * FH)
        Wq_s = Wq[rows].reshape(HPC, D, E)[:, perm, :].reshape(FH, E)
        Wk_s = Wk[rows].reshape(HPC, D, E)[:, perm, :].reshape(FH, E)
        bq_s = bq[rows].reshape(HPC, D)[:, perm]     # [HPC, 128]
        bk_s = bk[rows].reshape(HPC, D)[:, perm]
        bqk_t = PRJ * np.concatenate(
            [bq_s, bq_s[:, sw64], bk_s, bk_s[:, sw64]],
            axis=0).T.astype(np.float32)
        bqk_t = np.ascontiguousarray(bqk_t)          # [128, 4*HPC]

        wqh, wql = _split8(np.ascontiguousarray(Wq_s.T), SW)
        wkh, wkl = _split8(np.ascontiguousarray(Wk_s.T), SW)
        wvh, wvl = _split8(np.ascontiguousarray(Wv[rows].T), SW)
        woh, wol = _split8(np.ascontiguousarray(Wo[:, rows].T), SW)

        in_maps.append({
            "xhi": xsplit[b][0],
            "xlo": xsplit[b][1],
            "wqhi": _chunked(wqh, ECH), "wqlo": _chunked(wql, ECH),
            "wkhi": _chunked(wkh, ECH), "wklo": _chunked(wkl, ECH),
            "wvhi": _chunked(wvh, ECH), "wvlo": _chunked(wvl, ECH),
            "wohi": _chunked(woh, HPC), "wolo": _chunked(wol, HPC),
            "bqk": bqk_t,
            "bv_rep": np.ascontiguousarray(
                np.broadcast_to(bv[rows], (128, FH))).astype(BFNP),
            "cos_t": cos_t,
            "sin_t": sin_t,
            "cmask": cmask,
        })

    res = run_bass_kernel_spmd(nc, in_maps, list(range(NCORES)))
    outs = [res.results[c]["out"] for c in range(NCORES)]

    result = np.empty((B, S, E), dtype=np.float32)
    for b in range(B):
        acc = outs[GROUPS * b].astype(np.float32)
        for g in range(1, GROUPS):
            acc = acc + outs[GROUPS * b + g].astype(np.float32)
        result[b] = acc + bo[None, :]
    return result
